# revision 38
# baseline (speedup 1.0000x reference)
"""MixtureOfSoftMaxACF Trainium2 kernel.

Per-core (data-parallel over BS=8 across 8 cores, batch b per core):
  qt[b] memory reinterpreted as QQ[2, 2048, 64] (contiguous halves), same kt.
  For m in {0,1}:  S_m = QQ[m] @ KK[m].T / sqrt(128);  P_m = softmax(S_m, axis=-1)
  out[b] = (p0 * P_0 + p1 * P_1) @ vt[b]
  p: mixture prior (softmax over batch axis) -> computed on host, passed per-core.

Device pipeline per core:
  - Chunked staging: qt/kt arrive as 4-chunk groups (DMAs interleaved q/k),
    DVE-cast to bf16, PE-transposed per key-chunk into QT/KT [128, 2048] bf16
    slabs -- transposes overlap the remaining staging DMAs.
  - Scores (phase A, per qh half and mixture m): S^T [128 keys, 1024 q] =
    lhsT(K^T chunk [64,128]) @ rhs(Q^T slab) in bf16; ScalarE exp -> E bf16.
  - AV (phase B): E-STATIONARY: per 128-query block qb,
      O[q 128, 129] += E[c][:, qb]^T @ [V_c | ones]   (bf16, FWL weight loads)
    The ones column makes column 128 the softmax denominator D -- one extra
    moving column instead of a second matmul stream. Output lands in [q, dv].
  - Normalize/combine on DVE per-partition: res = sum_m (p_m/D_m[q]) * O_m.
  - Software pipelining: phase A of the next (qh, m) is interleaved into the
    current phase B (2 score chunks per query block) so the scalar engine's
    exp stream runs concurrently with the AV matmuls and PE never waits.
"""

import math
from contextlib import ExitStack

import numpy as np

import concourse.bass as bass
import concourse.bacc as bacc
import concourse.mybir as mybir
import concourse.tile as tile
from concourse.bass_utils import run_bass_kernel_spmd
from concourse.masks import make_identity

BS = 8
N = 2048          # queries
NK = 2048         # keys
DK = 128
M = 2
D = DK // M       # 64
DV = 128
TEMP = math.sqrt(DK)
NCH = NK // 128   # 16 key chunks
# staging groups (start_chunk, n_chunks) per input: small early groups land
# as parallel DMA queues (one dma_start ~ 27 GB/s; bandwidth = many queues)
SGROUPS = {
    "q": ((0, 2), (2, 2), (4, 2), (6, 2), (8, 4), (12, 4)),
    "k": ((0, 2), (2, 2), (4, 4), (8, 4), (12, 4)),
}
QH = 2            # query halves
QHN = N // QH     # 1024
NQB = QHN // 128  # 8 query blocks per half

F32 = mybir.dt.float32
BF16 = mybir.dt.bfloat16

_NC = None
LAST_RESULT = None  # BassKernelResults of last run (test.py reads this)


def _build():
    nc = bacc.Bacc(None)
    qt_d = nc.declare_dram_parameter("qt_b", [N, DK], F32, isOutput=False)
    kt_d = nc.declare_dram_parameter("kt_b", [NK, DK], F32, isOutput=False)
    vt_d = nc.declare_dram_parameter("vt_b", [NK, DK], F32, isOutput=False)
    pr_d = nc.declare_dram_parameter("pr_b", [1, M], F32, isOutput=False)
    out_d = nc.declare_dram_parameter("out_b", [N, DK], F32, isOutput=True)

    with ExitStack() as ctx:
        tc = ctx.enter_context(tile.TileContext(nc))
        const = ctx.enter_context(tc.tile_pool(name="const", bufs=1))
        sbig = ctx.enter_context(tc.tile_pool(name="sbig", bufs=1))
        epool = ctx.enter_context(tc.tile_pool(name="epool", bufs=3))
        npool = ctx.enter_context(tc.tile_pool(name="npool", bufs=2))
        ps_s = ctx.enter_context(tc.tile_pool(name="ps_s", bufs=2, space="PSUM"))
        ps_o = ctx.enter_context(tc.tile_pool(name="ps_o", bufs=2, space="PSUM"))

        # ---- constants (gpsimd, before anything queues on it) ----
        ident_b = const.tile([128, 128], BF16)
        make_identity(nc, ident_b)
        # Zero-padded K slabs: only the halves that stay zero are memset.
        qt_t = sbig.tile([128, N], BF16)
        kt_p0 = sbig.tile([128, NK], BF16)
        kt_p1 = sbig.tile([128, NK], BF16)
        kt_p = [kt_p0, kt_p1]
        nc.gpsimd.memset(kt_p0[D:128, :], 0.0)
        nc.gpsimd.memset(kt_p1[0:D, :], 0.0)

        # ---- staging DMAs in 4-chunk groups, q/k interleaved so the first
        # groups of both arrive early; issue alternates sync/scalar DMA queues
        # (a dma_start costs ~650ns of issue time, serial per engine).
        # stage[p, cl, m*64+d] = src[(g*G+cl)*128+p, m*64+d]
        srcs = {"q": qt_d, "k": kt_d}
        stg = {"q": [], "k": []}
        for name in ("q", "k"):
            for g, (c0, sz) in enumerate(SGROUPS[name]):
                t = sbig.tile([128, sz, DK], F32, tag=f"st_{name}{g}")
                stg[name].append(t)

        def stage_dma(eng, name, g, m):
            c0, sz = SGROUPS[name][g]
            eng.dma_start(
                out=stg[name][g][:, :, m * D:(m + 1) * D],
                in_=bass.AP(
                    tensor=srcs[name],
                    offset=m * N * D + c0 * 128 * D,
                    ap=[[D, 128], [128 * D, sz], [1, D]],
                ),
            )

        # Early wave: Q chunks 0-7 (all columns of the first scores) and
        # K chunks 0-3, m0 halves on sync / m1 on scalar -- 12 parallel queues.
        for g in range(4):
            stage_dma(nc.sync, "q", g, 0)
            stage_dma(nc.scalar, "q", g, 1)
        for g in range(2):
            stage_dma(nc.sync, "k", g, 0)
            stage_dma(nc.scalar, "k", g, 1)

        # Late K groups on gpsimd (behind its constant memsets -- their serial
        # issue staggers them after the early wave, landing just ahead of the
        # exp-paced score stream).
        for g in (2, 3, 4):
            for m in range(M):
                stage_dma(nc.gpsimd, "k", g, m)

        # Late sync wave, gated on the first Q group by a dummy SBUF->SBUF DMA
        # so V/Q-tail bandwidth doesn't starve the early wave.
        v_st = sbig.tile([128, NCH, DV], F32)
        pr_sb = const.tile([128, M], F32)
        v2 = sbig.tile([128, NCH, DV + 2], BF16)
        gate = const.tile([1, 8], F32)
        nc.sync.dma_start(out=gate, in_=stg["q"][0][0:1, 0, D - 4:D + 4])
        nc.sync.dma_start(
            out=pr_sb,
            in_=bass.AP(tensor=pr_d, offset=0, ap=[[0, 128], [1, M]]),
        )
        for p in range(4):
            nc.sync.dma_start(
                out=v_st[:, 4 * p:4 * (p + 1), :],
                in_=bass.AP(tensor=vt_d, offset=p * 4 * 128 * DK,
                            ap=[[DK, 128], [128 * DK, 4], [1, DV]]),
            )
        for g in (4, 5):
            for m in range(M):
                stage_dma(nc.sync, "q", g, m)

        # ---- bf16 casts of staging groups + per-chunk PE transposes ----
        # K goes into TWO zero-padded slabs (kt_p[m]: the other mixture's
        # 64 d-rows are zero) so score matmuls use full 128-contraction with
        # the same (128,128) tile geometry as the AV matmuls -- geometry
        # switches flush the PE pipeline.
        stb = {"q": [None] * len(SGROUPS["q"]), "k": [None] * len(SGROUPS["k"])}

        def group_of(name, c):
            for g, (c0, sz) in enumerate(SGROUPS[name]):
                if c0 <= c < c0 + sz:
                    return g
            raise AssertionError

        def ensure_cast(name, g):
            if stb[name][g] is None:
                sz = SGROUPS[name][g][1]
                b = sbig.tile([128, sz, DK], BF16, tag=f"stb_{name}{g}")
                nc.vector.tensor_copy(b, stg[name][g])
                stb[name][g] = b
            return stb[name][g]

        def emit_transpose(name, c):
            g = group_of(name, c)
            b = ensure_cast(name, g)
            # transposes borrow the AV-output PSUM buffers (bitcast to bf16)
            tp_o = ps_o.tile([128, DV + 1], F32, tag="O")
            tp = tp_o.bitcast(BF16)[:, 0:128]
            nc.tensor.transpose(tp, b[:, c - SGROUPS[name][g][0], :], ident_b)
            sl = slice(c * 128, (c + 1) * 128)
            if name == "q":
                nc.vector.tensor_copy(qt_t[:, sl], tp)
            else:
                nc.vector.tensor_copy(kt_p0[0:D, sl], tp[0:D, :])
                nc.vector.tensor_copy(kt_p1[D:128, sl], tp[D:128, :])

        # Q chunks 0..7 + K chunks 0..1 are all the first scores need;
        # later K chunks are prefetched 2 iterations ahead inside phase A.
        for c in range(4):
            emit_transpose("q", c)
        emit_transpose("k", 0)
        emit_transpose("k", 1)
        for c in range(4, 8):
            emit_transpose("q", c)

        # ---- emitters ----
        scale = 1.0 / TEMP
        # Score blocks (512 q cols each) are packed 3 per PSUM tile so one
        # ACTIVATE covers 1536 columns (amortizes the ~300ns ACT overhead).
        # E tensors are flat [128, NCH*QHN]; block i of stream (qh,m) holds
        # E columns [i*512, (i+1)*512) with i = 2*c + hf.
        sstate = {"tile": None, "n": 0, "E": None, "base": 0, "stream": None}

        def flush_scores():
            if sstate["tile"] is None:
                return
            n, base, E = sstate["n"], sstate["base"], sstate["E"]
            nc.scalar.activation(E[:, base * 512:(base + n) * 512],
                                 sstate["tile"][:, 0:n * 512],
                                 mybir.ActivationFunctionType.Exp, scale=scale)
            sstate["tile"] = None

        def emit_scores(qh, m, c, E):
            for hf in range(2):
                if sstate["tile"] is not None and sstate["stream"] != (qh, m):
                    flush_scores()
                if sstate["tile"] is None:
                    st = ps_s.tile([128, 3 * 512], F32, tag="s")
                    sstate.update(tile=st, n=0, E=E, base=2 * c + hf,
                                  stream=(qh, m))
                slot = sstate["n"]
                nc.tensor.matmul(
                    sstate["tile"][:, slot * 512:(slot + 1) * 512],
                    lhsT=kt_p[m][:, c * 128:(c + 1) * 128],
                    rhs=qt_t[:, qh * QHN + hf * 512: qh * QHN + (hf + 1) * 512],
                    start=True, stop=True,
                )
                sstate["n"] += 1
                if sstate["n"] == 3:
                    flush_scores()

        def emit_av(qh, m, qb, E, t0_list):
            O = ps_o.tile([128, DV + 1], F32, tag="O")
            for c in range(NCH):
                nc.tensor.matmul(
                    O,
                    lhsT=E[:, c * QHN + qb * 128: c * QHN + (qb + 1) * 128],
                    rhs=v2[:, c, 0:DV + 1],
                    start=(c == 0), stop=(c == NCH - 1),
                )
            d_r = npool.tile([128, 1], F32, tag="dr")
            nc.vector.reciprocal(d_r, O[:, DV:DV + 1])
            d_rp = npool.tile([128, 1], F32, tag="drp")
            nc.vector.tensor_scalar_mul(d_rp, d_r, pr_sb[:, m:m + 1])
            if m == 0:
                t = npool.tile([128, DV], F32, tag=f"t0_{qb}")
                nc.vector.tensor_scalar_mul(t, O[:, 0:DV], d_rp)
                t0_list.append(t)
            else:
                res = npool.tile([128, DV], F32, tag=f"res_{qb}")
                nc.vector.scalar_tensor_tensor(
                    out=res, in0=O[:, 0:DV], scalar=d_rp, in1=t0_list[qb],
                    op0=mybir.AluOpType.mult, op1=mybir.AluOpType.add,
                )
                nc.sync.dma_start(
                    out=bass.AP(
                        tensor=out_d,
                        offset=(qh * QHN + qb * 128) * DK,
                        ap=[[DK, 128], [1, DV]],
                    ),
                    in_=res,
                )

        # ---- phase A(0,0), with remaining transposes interleaved ----
        E_cur = {}

        def get_E(st):
            if st not in E_cur:
                Enew = epool.tile([128, NCH * QHN], BF16, tag="E")
                E_cur[st] = Enew
            return E_cur[st]

        for c in range(NCH):
            if c + 2 < NCH:
                emit_transpose("k", c + 2)
            emit_scores(0, 0, c, get_E((0, 0)))
            if c == 0:
                flush_scores()  # start the ACT exp stream as early as possible
            if c >= 8:
                emit_transpose("q", c)
        flush_scores()

        # v2 build rides in DVE slack during phase A (needed at first AV).
        nc.vector.memset(v2, 1.0)
        nc.vector.tensor_copy(v2[:, :, 0:DV], v_st)

        # ---- interleaved B/S schedule: remaining score streams ride inside
        # the AV phases, 3 chunks per query block, front-loaded so the last
        # AV phases run exp-free at full PE rate ----
        feeder = [(qh, m, c) for (qh, m) in ((0, 1), (1, 0), (1, 1))
                  for c in range(NCH)]
        fi = [0]
        t0_store = {0: [], 1: []}
        for bqh, bm in ((0, 0), (0, 1), (1, 0), (1, 1)):
            for qb in range(NQB):
                emit_av(bqh, bm, qb, get_E((bqh, bm)), t0_store[bqh])
                for _ in range(3):
                    if fi[0] < len(feeder):
                        qh2, m2, c2 = feeder[fi[0]]
                        fi[0] += 1
                        emit_scores(qh2, m2, c2, get_E((qh2, m2)))
                        if fi[0] == len(feeder):
                            flush_scores()
    return nc


def _get_nc():
    global _NC
    if _NC is None:
        _NC = _build()
        _NC.finalize()  # Bacc.compile(): event sems, reg alloc, wait legalization
    return _NC


def _prior(qt, kernel):
    bar_qt = qt.astype(np.float32).mean(axis=1)          # (BS, dk)
    logits = kernel.astype(np.float32) @ bar_qt.T        # (m, BS)
    z = logits - logits.max(axis=1, keepdims=True)
    ez = np.exp(z)
    pm = ez / ez.sum(axis=1, keepdims=True)              # softmax over batch axis
    return pm.reshape(-1)


def kernel(qt, kt, vt, kernel):
    global LAST_RESULT
    import os
    nc = _get_nc()
    prior_flat = _prior(qt, kernel)
    in_maps = []
    for b in range(BS):
        pr = np.array([[prior_flat[2 * b], prior_flat[2 * b + 1]]], dtype=np.float32)
        in_maps.append({
            "qt_b": np.ascontiguousarray(qt[b], dtype=np.float32),
            "kt_b": np.ascontiguousarray(kt[b], dtype=np.float32),
            "vt_b": np.ascontiguousarray(vt[b], dtype=np.float32),
            "pr_b": pr,
        })
    trace = bool(int(os.environ.get("KERNEL_TRACE", "0")))
    res = run_bass_kernel_spmd(nc, in_maps, list(range(BS)), trace=trace)
    LAST_RESULT = res
    out = np.stack([np.asarray(res.results[b]["out_b"]).reshape(N, DK) for b in range(BS)])
    return out.astype(np.float32)
